# revision 10
# baseline (speedup 1.0000x reference)
"""Trainium2 Bass kernel for AtomEmbedding:
    h = LayerNorm(emb[z] + W2 @ silu(W1 @ x + b1) + b2) * gamma + beta

Strategy (pure data parallel over the packed atom axis):
  - N = 524288 atoms sharded 65536/core over 8 NeuronCores; all params replicated.
  - All matmuls run on the PE in float32r (full-rate for moving dim >= 256).
  - mm1 computes p^T = w1_aug^T @ x_aug^T in D-on-partitions layout so that
    silu(p)^T can be used directly as the stationary operand of mm2
    (no transposes anywhere). b1 is folded in via an ones-row in x_aug.
  - The embedding gather is a one-hot matmul accumulated into the same PSUM
    tile as mm2, so h = emb'[z] + p forms entirely inside PSUM (b2 is folded
    into emb' = emb + b2 host-side). The one-hot (types x atoms) is built by
    broadcasting z across 100 partitions (gpsimd partition_broadcast) and
    comparing against a per-partition iota column on the DVE.
  - LayerNorm per 128-atom tile: bn_stats/bn_aggr (one DVE pass over PSUM),
    sqrt(var+eps) on ACT + reciprocal on DVE, then a fused dual-scalar
    tensor_scalar (h - mu) * rstd. gamma/beta are applied only when they are
    not the trivial ones/zeros (they are trivial for this problem's inputs).
"""

import os
import sys

import numpy as np

for _p in ("/opt/trn_rl_repo", "/opt/pypackages"):
    if _p not in sys.path and os.path.isdir(_p):
        sys.path.append(_p)

N = 524288
D = 256
NT = 100  # number of atom types
NCORES = 8
NPC = N // NCORES  # atoms per core
A = 512  # atoms per group (one moving-operand pass)
TPG = A // 128  # 128-atom tiles per group
EPS = 1e-5

# matmul operand dtype: "f32r" (fp32 storage, fast PE mode) or "bf16"
MM_MODE = os.environ.get("ATOMEMB_MM_MODE", "f32r")

_MODULE_CACHE: dict = {}


def _build_module(npc: int, apply_affine: bool, mm_mode: str,
                  sim_safe_silu: bool = False):
    """Build + compile the Bass module for one core's slice (npc atoms).

    sim_safe_silu: CoreSim doesn't implement the Silu activation; when True,
    emit Sigmoid + multiply instead (slower, only used for simulation runs).
    """
    from contextlib import ExitStack

    import concourse.bacc as bacc
    import concourse.tile as tile
    from concourse import mybir

    f32 = mybir.dt.float32
    if mm_mode == "f32r":
        # float32r tiles end-to-end: every producer of a matmul operand
        # (DMA from an f32r DRAM tensor, ACT silu, DVE compare) then counts
        # as rounding to f32r for the BIR verifier.
        st_dt = mybir.dt.float32r
    elif mm_mode == "bf16":
        st_dt = mybir.dt.bfloat16
    else:
        raise ValueError(mm_mode)

    ngroups = npc // A

    nc = bacc.Bacc(
        "TRN2",
        target_bir_lowering=False,
        debug=False,
        enable_asserts=False,
        num_devices=NCORES,
    )

    # Per-core inputs (host pre-transposed / folded):
    #   xT:   [4, npc]  rows = (x0, x1, x2, 1)          -> moving operand of mm1
    #   zrow: [1, npc]  z as float
    #   w1c:  [4, D]    [w1; b1]                        -> stationary of mm1
    #   w2a:  [2, 128, D] w2 split into two k-chunks    -> moving of mm2
    #   emba: [NT, D]   emb + b2                        -> moving of gather-mm
    #   iotac:[NT, 1]   0..NT-1 column
    xT = nc.dram_tensor("xT", [4, npc], st_dt, kind="ExternalInput")
    zrow = nc.dram_tensor("zrow", [1, npc], st_dt, kind="ExternalInput")
    w1c = nc.dram_tensor("w1c", [4, D], st_dt, kind="ExternalInput")
    w2a = nc.dram_tensor("w2a", [128, 2, D], st_dt, kind="ExternalInput")
    emba = nc.dram_tensor("emba", [NT, D], st_dt, kind="ExternalInput")
    iotac = nc.dram_tensor("iotac", [NT, 1], f32, kind="ExternalInput")
    if apply_affine:
        gmb = nc.dram_tensor("gmb", [128, D], f32, kind="ExternalInput")
        btb = nc.dram_tensor("btb", [128, D], f32, kind="ExternalInput")
    out = nc.dram_tensor("out", [npc, D], f32, kind="ExternalOutput")

    def mm(ap):
        return ap

    with tile.TileContext(nc) as tc:
        with ExitStack() as ctx:
            consts = ctx.enter_context(tc.tile_pool(name="consts", bufs=1))
            xpool = ctx.enter_context(tc.tile_pool(name="xpool", bufs=3))
            zpool = ctx.enter_context(tc.tile_pool(name="zpool", bufs=3))
            zbpool = ctx.enter_context(tc.tile_pool(name="zbpool", bufs=2))
            ohpool = ctx.enter_context(tc.tile_pool(name="ohpool", bufs=2))
            spool = ctx.enter_context(tc.tile_pool(name="spool", bufs=4))
            stpool = ctx.enter_context(tc.tile_pool(name="stpool", bufs=12))
            opool = ctx.enter_context(tc.tile_pool(name="opool", bufs=6))
            psA = ctx.enter_context(tc.tile_pool(name="psA", bufs=2, space="PSUM"))
            psB = ctx.enter_context(tc.tile_pool(name="psB", bufs=4, space="PSUM"))

            # ---- load constants once ----
            sb_w1 = consts.tile([4, D], st_dt)
            nc.sync.dma_start(out=sb_w1[:], in_=w1c[:])
            sb_w2 = consts.tile([128, 2, D], st_dt)
            nc.sync.dma_start(out=sb_w2[:], in_=w2a[:])
            sb_emb = consts.tile([NT, D], st_dt)
            nc.sync.dma_start(out=sb_emb[:], in_=emba[:])
            sb_iota = consts.tile([NT, 1], f32)
            nc.sync.dma_start(out=sb_iota[:], in_=iotac[:])
            sb_eps = consts.tile([128, 1], f32)
            nc.vector.memset(sb_eps[:], EPS)
            if apply_affine:
                sb_gmb = consts.tile([128, D], f32)
                nc.sync.dma_start(out=sb_gmb[:], in_=gmb[:])
                sb_btb = consts.tile([128, D], f32)
                nc.sync.dma_start(out=sb_btb[:], in_=btb[:])

            for g in range(ngroups):
                a0 = g * A
                # ---- loads ----
                xt = xpool.tile([4, A], st_dt, tag="xt")
                nc.sync.dma_start(out=xt[:], in_=xT[:, a0 : a0 + A])
                zt = zpool.tile([1, A], st_dt, tag="zt")
                nc.sync.dma_start(out=zt[:], in_=zrow[:, a0 : a0 + A])

                # ---- one-hot^T [NT, A]: broadcast z, compare to iota ----
                zb = zbpool.tile([NT, A], st_dt, tag="zb")
                nc.gpsimd.partition_broadcast(zb[:], zt[:], channels=NT)
                oh = ohpool.tile([NT, A], st_dt, tag="oh")
                nc.vector.tensor_scalar(
                    out=oh[:],
                    in0=zb[:],
                    scalar1=sb_iota[:],
                    scalar2=None,
                    op0=mybir.AluOpType.is_equal,
                )

                # ---- mm1: p^T chunks [128, A] (D on partitions) ----
                pT0 = psA.tile([128, A], f32, tag="pT0")
                pT1 = psA.tile([128, A], f32, tag="pT1")
                nc.tensor.matmul(pT0[:], mm(sb_w1[:, 0:128]), mm(xt[:]),
                                 start=True, stop=True)
                nc.tensor.matmul(pT1[:], mm(sb_w1[:, 128:256]), mm(xt[:]),
                                 start=True, stop=True)

                # ---- silu on ACT: s^T = Silu(p^T) ----
                s0 = spool.tile([128, A], st_dt, tag="s0")
                s1 = spool.tile([128, A], st_dt, tag="s1")
                if sim_safe_silu:
                    for ps, s in ((pT0, s0), (pT1, s1)):
                        sg = spool.tile([128, A], f32, tag="sg")
                        nc.scalar.activation(sg[:], ps[:],
                                             mybir.ActivationFunctionType.Sigmoid)
                        nc.vector.tensor_mul(s[:], sg[:], ps[:])
                else:
                    nc.scalar.activation(s0[:], pT0[:],
                                         mybir.ActivationFunctionType.Silu)
                    nc.scalar.activation(s1[:], pT1[:],
                                         mybir.ActivationFunctionType.Silu)

                for t in range(TPG):
                    c = t * 128
                    # ---- mm2 + embedding gather accumulate: h in PSUM ----
                    h = psB.tile([128, D], f32, tag="h")
                    nc.tensor.matmul(h[:], mm(s0[:, c : c + 128]), mm(sb_w2[:, 0, :]),
                                     start=True, stop=False)
                    nc.tensor.matmul(h[:], mm(s1[:, c : c + 128]), mm(sb_w2[:, 1, :]),
                                     start=False, stop=False)
                    nc.tensor.matmul(h[:], mm(oh[:, c : c + 128]), mm(sb_emb[:]),
                                     start=False, stop=True)

                    # ---- LayerNorm ----
                    st = stpool.tile([128, 6], f32, tag="st")
                    nc.vector.bn_stats(out=st[:], in_=h[:])
                    mv = stpool.tile([128, 2], f32, tag="mv")
                    nc.vector.bn_aggr(out=mv[:], in_=st[:])
                    sd = stpool.tile([128, 1], f32, tag="sd")
                    nc.scalar.activation(sd[:], mv[:, 1:2],
                                         mybir.ActivationFunctionType.Sqrt,
                                         bias=sb_eps[:])
                    rs = stpool.tile([128, 1], f32, tag="rs")
                    nc.vector.reciprocal(out=rs[:], in_=sd[:])

                    o = opool.tile([128, D], f32, tag="o")
                    nc.vector.tensor_scalar(
                        out=o[:],
                        in0=h[:],
                        scalar1=mv[:, 0:1],
                        scalar2=rs[:],
                        op0=mybir.AluOpType.subtract,
                        op1=mybir.AluOpType.mult,
                    )
                    if apply_affine:
                        nc.vector.tensor_mul(o[:], o[:], sb_gmb[:])
                        nc.vector.tensor_add(o[:], o[:], sb_btb[:])

                    nc.sync.dma_start(out=out[a0 + c : a0 + c + 128, :], in_=o[:])

    nc.compile()
    return nc


def _get_module(npc: int, apply_affine: bool, mm_mode: str = MM_MODE,
                sim_safe_silu: bool = False):
    key = (npc, apply_affine, mm_mode, sim_safe_silu)
    if key not in _MODULE_CACHE:
        _MODULE_CACHE[key] = _build_module(npc, apply_affine, mm_mode,
                                           sim_safe_silu)
    return _MODULE_CACHE[key]


def _prep_inputs(z, x, emb, w1, b1, w2, b2, gamma, beta, npc, apply_affine,
                 mm_mode: str = MM_MODE):
    """Host-side folding/transposes; returns per-core in_maps."""
    if mm_mode == "f32r":
        st = np.float32
    else:
        import ml_dtypes

        st = ml_dtypes.bfloat16

    z = np.asarray(z)
    x = np.asarray(x, dtype=np.float32)
    n = z.shape[0]

    xT = np.empty((4, n), dtype=np.float32)
    xT[0:3] = x.T
    xT[3] = 1.0
    xT = xT.astype(st)
    zrow = np.asarray(z, dtype=np.float32).reshape(1, n).astype(st)
    w1a = np.concatenate([np.asarray(w1, np.float32),
                          np.asarray(b1, np.float32).reshape(1, D)], axis=0)
    w1c = w1a.astype(st)
    w2f = np.asarray(w2, np.float32)
    w2a = np.stack([w2f[0:128], w2f[128:256]], axis=1).astype(st)
    emba = (np.asarray(emb, np.float32)
            + np.asarray(b2, np.float32).reshape(1, D)).astype(st)
    iotac = np.arange(NT, dtype=np.float32).reshape(NT, 1)

    common = {"w1c": w1c, "w2a": w2a, "emba": emba, "iotac": iotac}
    if apply_affine:
        common["gmb"] = np.broadcast_to(
            np.asarray(gamma, np.float32).reshape(1, D), (128, D)).copy()
        common["btb"] = np.broadcast_to(
            np.asarray(beta, np.float32).reshape(1, D), (128, D)).copy()

    in_maps = []
    for c in range(NCORES):
        s = slice(c * npc, (c + 1) * npc)
        in_maps.append({"xT": np.ascontiguousarray(xT[:, s]),
                        "zrow": np.ascontiguousarray(zrow[:, s]),
                        **common})
    return in_maps


def _run(in_maps, nc, trace=False):
    from concourse.bass_interp import get_hw_module
    from concourse.bass_utils import run_bass_kernel_spmd

    old_m = nc.m
    nc.m = get_hw_module(nc.m)
    try:
        res = run_bass_kernel_spmd(
            nc, in_maps, core_ids=list(range(NCORES)), trace=trace
        )
    finally:
        nc.m = old_m
    return res


def kernel(z, x, emb, w1, b1, w2, b2, gamma, beta):
    z = np.asarray(z)
    x = np.asarray(x)
    assert z.shape[0] == N and x.shape == (N, 3), (z.shape, x.shape)

    apply_affine = not (
        np.all(np.asarray(gamma) == 1.0) and np.all(np.asarray(beta) == 0.0)
    )
    nc = _get_module(NPC, apply_affine)
    in_maps = _prep_inputs(z, x, emb, w1, b1, w2, b2, gamma, beta,
                           NPC, apply_affine)
    res = _run(in_maps, nc, trace=False)
    out = np.concatenate([r["out"] for r in res.results], axis=0)
    return out.astype(np.float32)


# revision 11
# speedup vs baseline: 35.9312x; 35.9312x over previous
"""Trainium2 Bass kernel for AtomEmbedding:
    h = LayerNorm(emb[z] + W2 @ silu(W1 @ x + b1) + b2) * gamma + beta

Strategy (pure data parallel over the packed atom axis):
  - N = 524288 atoms sharded 65536/core over 8 NeuronCores; all params replicated.
  - All matmuls run on the PE in float32r (full-rate for moving dim >= 256).
  - mm1 computes p^T = w1_aug^T @ x_aug^T in D-on-partitions layout so that
    silu(p)^T can be used directly as the stationary operand of mm2
    (no transposes anywhere). b1 is folded in via an ones-row in x_aug.
  - The embedding gather is a one-hot matmul accumulated into the same PSUM
    tile as mm2, so h = emb'[z] + p forms entirely inside PSUM (b2 is folded
    into emb' = emb + b2 host-side). The one-hot (types x atoms) is built by
    broadcasting z across 100 partitions (gpsimd partition_broadcast) and
    comparing against a per-partition iota column on the DVE.
  - LayerNorm per 128-atom tile: bn_stats/bn_aggr (one DVE pass over PSUM),
    sqrt(var+eps) on ACT + reciprocal on DVE, then a fused dual-scalar
    tensor_scalar (h - mu) * rstd. gamma/beta are applied only when they are
    not the trivial ones/zeros (they are trivial for this problem's inputs).
"""

import os
import sys

import numpy as np

for _p in ("/opt/trn_rl_repo", "/opt/pypackages"):
    if _p not in sys.path and os.path.isdir(_p):
        sys.path.append(_p)

N = 524288
D = 256
NT = 100  # number of atom types
NCORES = 8
NPC = N // NCORES  # atoms per core
A = 512  # atoms per group (one moving-operand pass)
TPG = A // 128  # 128-atom tiles per group
EPS = 1e-5

# matmul operand dtype: "f32r" (fp32 storage, fast PE mode) or "bf16"
MM_MODE = os.environ.get("ATOMEMB_MM_MODE", "f32r")

_MODULE_CACHE: dict = {}


def _build_module(npc: int, apply_affine: bool, mm_mode: str,
                  sim_safe_silu: bool = False):
    """Build + compile the Bass module for one core's slice (npc atoms).

    sim_safe_silu: CoreSim doesn't implement the Silu activation; when True,
    emit Sigmoid + multiply instead (slower, only used for simulation runs).
    """
    from contextlib import ExitStack

    import concourse.bacc as bacc
    import concourse.tile as tile
    from concourse import mybir

    f32 = mybir.dt.float32
    if mm_mode == "f32r":
        # float32r tiles end-to-end: every producer of a matmul operand
        # (DMA from an f32r DRAM tensor, ACT silu, DVE compare) then counts
        # as rounding to f32r for the BIR verifier.
        st_dt = mybir.dt.float32r
    elif mm_mode == "bf16":
        st_dt = mybir.dt.bfloat16
    else:
        raise ValueError(mm_mode)

    ngroups = npc // A

    nc = bacc.Bacc(
        "TRN2",
        target_bir_lowering=False,
        debug=False,
        enable_asserts=False,
        num_devices=NCORES,
    )

    # Per-core inputs (host pre-transposed / folded):
    #   xT:   [4, npc]  rows = (x0, x1, x2, 1)          -> moving operand of mm1
    #   zrow: [1, npc]  z as float
    #   w1c:  [4, D]    [w1; b1]                        -> stationary of mm1
    #   w2a:  [2, 128, D] w2 split into two k-chunks    -> moving of mm2
    #   emba: [NT, D]   emb + b2                        -> moving of gather-mm
    #   iotac:[NT, 1]   0..NT-1 column
    xT = nc.dram_tensor("xT", [4, npc], st_dt, kind="ExternalInput")
    zrow = nc.dram_tensor("zrow", [1, npc], st_dt, kind="ExternalInput")
    w1c = nc.dram_tensor("w1c", [4, D], st_dt, kind="ExternalInput")
    w2a = nc.dram_tensor("w2a", [128, 2, D], st_dt, kind="ExternalInput")
    emba = nc.dram_tensor("emba", [NT, D], st_dt, kind="ExternalInput")
    iotac = nc.dram_tensor("iotac", [NT, 1], f32, kind="ExternalInput")
    if apply_affine:
        gmb = nc.dram_tensor("gmb", [128, D], f32, kind="ExternalInput")
        btb = nc.dram_tensor("btb", [128, D], f32, kind="ExternalInput")
    out = nc.dram_tensor("out", [npc, D], f32, kind="ExternalOutput")

    def mm(ap):
        return ap

    with tile.TileContext(nc) as tc:
        with ExitStack() as ctx:
            consts = ctx.enter_context(tc.tile_pool(name="consts", bufs=1))
            xpool = ctx.enter_context(tc.tile_pool(name="xpool", bufs=3))
            zpool = ctx.enter_context(tc.tile_pool(name="zpool", bufs=3))
            zbpool = ctx.enter_context(tc.tile_pool(name="zbpool", bufs=2))
            ohpool = ctx.enter_context(tc.tile_pool(name="ohpool", bufs=2))
            spool = ctx.enter_context(tc.tile_pool(name="spool", bufs=4))
            stpool = ctx.enter_context(tc.tile_pool(name="stpool", bufs=12))
            opool = ctx.enter_context(tc.tile_pool(name="opool", bufs=6))
            psA = ctx.enter_context(tc.tile_pool(name="psA", bufs=2, space="PSUM"))
            psB = ctx.enter_context(tc.tile_pool(name="psB", bufs=4, space="PSUM"))

            # ---- load constants once ----
            sb_w1 = consts.tile([4, D], st_dt)
            nc.sync.dma_start(out=sb_w1[:], in_=w1c[:])
            sb_w2 = consts.tile([128, 2, D], st_dt)
            nc.sync.dma_start(out=sb_w2[:], in_=w2a[:])
            sb_emb = consts.tile([NT, D], st_dt)
            nc.sync.dma_start(out=sb_emb[:], in_=emba[:])
            sb_iota = consts.tile([NT, 1], f32)
            nc.sync.dma_start(out=sb_iota[:], in_=iotac[:])
            sb_eps = consts.tile([128, 1], f32)
            nc.vector.memset(sb_eps[:], EPS)
            if apply_affine:
                sb_gmb = consts.tile([128, D], f32)
                nc.sync.dma_start(out=sb_gmb[:], in_=gmb[:])
                sb_btb = consts.tile([128, D], f32)
                nc.sync.dma_start(out=sb_btb[:], in_=btb[:])

            for g in range(ngroups):
                a0 = g * A
                # ---- loads ----
                xt = xpool.tile([4, A], st_dt, tag="xt")
                nc.sync.dma_start(out=xt[:], in_=xT[:, a0 : a0 + A])
                zt = zpool.tile([1, A], st_dt, tag="zt")
                nc.sync.dma_start(out=zt[:], in_=zrow[:, a0 : a0 + A])

                # ---- one-hot^T [NT, A]: broadcast z, compare to iota ----
                zb = zbpool.tile([NT, A], st_dt, tag="zb")
                nc.gpsimd.partition_broadcast(zb[:], zt[:], channels=NT)
                oh = ohpool.tile([NT, A], st_dt, tag="oh")
                nc.vector.tensor_scalar(
                    out=oh[:],
                    in0=zb[:],
                    scalar1=sb_iota[:],
                    scalar2=None,
                    op0=mybir.AluOpType.is_equal,
                )

                # ---- mm1: p^T chunks [128, A] (D on partitions) ----
                pT0 = psA.tile([128, A], f32, tag="pT0")
                pT1 = psA.tile([128, A], f32, tag="pT1")
                nc.tensor.matmul(pT0[:], mm(sb_w1[:, 0:128]), mm(xt[:]),
                                 start=True, stop=True)
                nc.tensor.matmul(pT1[:], mm(sb_w1[:, 128:256]), mm(xt[:]),
                                 start=True, stop=True)

                # ---- silu on ACT: s^T = Silu(p^T) ----
                s0 = spool.tile([128, A], st_dt, tag="s0")
                s1 = spool.tile([128, A], st_dt, tag="s1")
                if sim_safe_silu:
                    for ps, s in ((pT0, s0), (pT1, s1)):
                        sg = spool.tile([128, A], f32, tag="sg")
                        nc.scalar.activation(sg[:], ps[:],
                                             mybir.ActivationFunctionType.Sigmoid)
                        nc.vector.tensor_mul(s[:], sg[:], ps[:])
                else:
                    nc.scalar.activation(s0[:], pT0[:],
                                         mybir.ActivationFunctionType.Silu)
                    nc.scalar.activation(s1[:], pT1[:],
                                         mybir.ActivationFunctionType.Silu)

                # ---- mm2 + embedding gather accumulate: h pairs in PSUM ----
                # Two 128-atom tiles share one PSUM bank so bn_stats can
                # process them in a single pass.
                hp = []
                for p in range(TPG // 2):
                    h2 = psB.tile([128, 2, D], f32, tag="h")
                    hp.append(h2)
                    for j in range(2):
                        c = (2 * p + j) * 128
                        nc.tensor.matmul(h2[:, j, :], mm(s0[:, c : c + 128]),
                                         mm(sb_w2[:, 0, :]), start=True, stop=False)
                        nc.tensor.matmul(h2[:, j, :], mm(s1[:, c : c + 128]),
                                         mm(sb_w2[:, 1, :]), start=False, stop=False)
                        nc.tensor.matmul(h2[:, j, :], mm(oh[:, c : c + 128]),
                                         mm(sb_emb[:]), start=False, stop=True)

                # ---- LayerNorm stats: one bn_stats per pair, batched tail ----
                stg = stpool.tile([128, TPG, 6], f32, tag="stg")
                for p in range(TPG // 2):
                    nc.vector.bn_stats(out=stg[:, 2 * p : 2 * p + 2, :],
                                       in_=hp[p][:])
                mvg = stpool.tile([128, TPG, 2], f32, tag="mvg")
                for t in range(TPG):
                    nc.vector.bn_aggr(out=mvg[:, t, :], in_=stg[:, t, :])
                # sd = sqrt(var + eps), rs = 1/sd, cc = -mu*rs  (all [128, TPG])
                sd = stpool.tile([128, TPG], f32, tag="sd")
                nc.scalar.activation(sd[:], mvg[:, :, 1],
                                     mybir.ActivationFunctionType.Sqrt,
                                     bias=sb_eps[:])
                rs = stpool.tile([128, TPG], f32, tag="rs")
                nc.vector.reciprocal(out=rs[:], in_=sd[:])
                cc = stpool.tile([128, TPG], f32, tag="cc")
                nc.vector.scalar_tensor_tensor(
                    out=cc[:], in0=mvg[:, :, 0], scalar=-1.0, in1=rs[:],
                    op0=mybir.AluOpType.mult, op1=mybir.AluOpType.mult,
                )

                # ---- normalize; split tiles between ACT and DVE ----
                for t in range(TPG):
                    h = hp[t // 2][:, t % 2, :]
                    c = t * 128
                    o = opool.tile([128, D], f32, tag="o")
                    if t % 4 < 2:
                        # ACT: o = h*rs + (-mu*rs)
                        nc.scalar.activation(
                            o[:], h,
                            mybir.ActivationFunctionType.Identity,
                            bias=cc[:, t : t + 1], scale=rs[:, t : t + 1])
                    else:
                        # DVE: o = (h - mu)*rs
                        nc.vector.tensor_scalar(
                            out=o[:], in0=h,
                            scalar1=mvg[:, t, 0:1], scalar2=rs[:, t : t + 1],
                            op0=mybir.AluOpType.subtract,
                            op1=mybir.AluOpType.mult,
                        )
                    if apply_affine:
                        nc.vector.tensor_mul(o[:], o[:], sb_gmb[:])
                        nc.vector.tensor_add(o[:], o[:], sb_btb[:])

                    nc.sync.dma_start(out=out[a0 + c : a0 + c + 128, :], in_=o[:])

    nc.compile()
    return nc


def _get_module(npc: int, apply_affine: bool, mm_mode: str = MM_MODE,
                sim_safe_silu: bool = False):
    key = (npc, apply_affine, mm_mode, sim_safe_silu)
    if key not in _MODULE_CACHE:
        _MODULE_CACHE[key] = _build_module(npc, apply_affine, mm_mode,
                                           sim_safe_silu)
    return _MODULE_CACHE[key]


def _prep_inputs(z, x, emb, w1, b1, w2, b2, gamma, beta, npc, apply_affine,
                 mm_mode: str = MM_MODE):
    """Host-side folding/transposes; returns per-core in_maps."""
    if mm_mode == "f32r":
        st = np.float32
    else:
        import ml_dtypes

        st = ml_dtypes.bfloat16

    z = np.asarray(z)
    x = np.asarray(x, dtype=np.float32)
    n = z.shape[0]

    xT = np.empty((4, n), dtype=np.float32)
    xT[0:3] = x.T
    xT[3] = 1.0
    xT = xT.astype(st)
    zrow = np.asarray(z, dtype=np.float32).reshape(1, n).astype(st)
    w1a = np.concatenate([np.asarray(w1, np.float32),
                          np.asarray(b1, np.float32).reshape(1, D)], axis=0)
    w1c = w1a.astype(st)
    w2f = np.asarray(w2, np.float32)
    w2a = np.stack([w2f[0:128], w2f[128:256]], axis=1).astype(st)
    emba = (np.asarray(emb, np.float32)
            + np.asarray(b2, np.float32).reshape(1, D)).astype(st)
    iotac = np.arange(NT, dtype=np.float32).reshape(NT, 1)

    common = {"w1c": w1c, "w2a": w2a, "emba": emba, "iotac": iotac}
    if apply_affine:
        common["gmb"] = np.broadcast_to(
            np.asarray(gamma, np.float32).reshape(1, D), (128, D)).copy()
        common["btb"] = np.broadcast_to(
            np.asarray(beta, np.float32).reshape(1, D), (128, D)).copy()

    in_maps = []
    for c in range(NCORES):
        s = slice(c * npc, (c + 1) * npc)
        in_maps.append({"xT": np.ascontiguousarray(xT[:, s]),
                        "zrow": np.ascontiguousarray(zrow[:, s]),
                        **common})
    return in_maps


def _run(in_maps, nc, trace=False):
    from concourse.bass_interp import get_hw_module
    from concourse.bass_utils import run_bass_kernel_spmd

    old_m = nc.m
    nc.m = get_hw_module(nc.m)
    try:
        res = run_bass_kernel_spmd(
            nc, in_maps, core_ids=list(range(NCORES)), trace=trace
        )
    finally:
        nc.m = old_m
    return res


def kernel(z, x, emb, w1, b1, w2, b2, gamma, beta):
    z = np.asarray(z)
    x = np.asarray(x)
    assert z.shape[0] == N and x.shape == (N, 3), (z.shape, x.shape)

    apply_affine = not (
        np.all(np.asarray(gamma) == 1.0) and np.all(np.asarray(beta) == 0.0)
    )
    nc = _get_module(NPC, apply_affine)
    in_maps = _prep_inputs(z, x, emb, w1, b1, w2, b2, gamma, beta,
                           NPC, apply_affine)
    res = _run(in_maps, nc, trace=False)
    out = np.concatenate([r["out"] for r in res.results], axis=0)
    return out.astype(np.float32)
